# revision 9
# baseline (speedup 1.0000x reference)
"""CRF Viterbi decode on 8 Trainium2 NeuronCores.

Strategy: time-sliced data parallelism over 256 "virtual lanes".
  - The 64 sequences' forward Viterbi recurrences are cut (at runtime,
    from the mask) into <=256 contiguous time-pieces.  Each piece runs
    in one lane: 8 cores x 32 lanes/core, every lane a length-N forward
    chain (N ~ 85 instead of 511).  Pieces that start mid-sequence get a
    short speculative burn-in prefix; Viterbi argmax decisions coalesce
    within a few steps, after which the piece's partition vector equals
    the true one up to a constant + O(ulp) dust.
  - Drift-kill: the host subtracts max_j feats[b,t,j] per step from the
    features, keeping partition values O(10) instead of O(500) so fp32
    dust stays ~1e-5 and near-tie flips are essentially impossible.
  - The host pre-adds transitions: FT[i,tau,j] = fl(feat'+trans), so the
    device program is pure linear DMA + the 2-op/step DVE chain:
        cur  = fl(FT[t] + part_{t-1})           (scalar_tensor_tensor)
        part = max_i cur   (tensor_reduce, 32x32-block apply_transpose)
    with per-core layout partitions=(pg:4, i:32), free=(g:8, j:32).
  - Host reassembles alpha, checks seam coalescence, backtracks exactly
    like the reference, flags any decision whose top-2 gap is within
    dust range (tau=2e-4), and recomputes flagged sequences exactly.
    On non-degenerate inputs zero or a handful of sequences get flagged.
"""

import numpy as np

B, S, T = 64, 512, 32
NCORES = 8
P = 128
START, END = T - 2, T - 1
W = 32            # lanes (pieces) per core
G = 8             # free-dim lane groups
PG = 4            # partition lane groups (PG * G == W)
BURN = 12         # speculative burn-in steps
CH0, CH = 8, 24   # first / steady chunk of time-steps for DMA+output
TAU_BP = 2e-4     # near-tie flag threshold on backtrack decisions
TAU_SEAM = 5e-4   # seam coalescence threshold (excluding START column)

_PROGRAM_CACHE = {}


def _chunks(n, first=CH0, step=CH):
    out, lo = [], 0
    while lo < n:
        hi = min(n, lo + (first if lo == 0 else step))
        if n - hi < step // 3:  # merge tiny tail into last chunk
            hi = n
        out.append((lo, hi))
        lo = hi
    return out


def _build_program(N):
    import concourse.mybir as mybir
    from concourse import bacc, tile

    AL = mybir.AluOpType
    F32 = mybir.dt.float32
    X = mybir.AxisListType.X

    nc = bacc.Bacc("TRN2", target_bir_lowering=False, debug=False)
    ftp_d = nc.dram_tensor("ftp", [PG, T, N, G, T], F32, kind="ExternalInput").ap()
    p0_d = nc.dram_tensor("part0", [W, T], F32, kind="ExternalInput").ap()
    out_d = nc.dram_tensor("parthist", [P, N * G], F32, kind="ExternalOutput").ap()

    with tile.TileContext(nc) as tc:
        with (
            tc.tile_pool(name="work", bufs=2) as wpool,
            tc.tile_pool(name="ft", bufs=1) as ftpool,
        ):
            parthist = ftpool.tile([P, N * G], F32, tag="parthist")
            ft = ftpool.tile([P, N * G * T], F32, tag="ft")

            # part0[(pg,j), g] = fl(lanefeat[pg*8+g, 0, j] + trans[START, j])
            for pg in range(PG):
                nc.sync.dma_start(
                    parthist[pg * 32:(pg + 1) * 32, 0:G],
                    p0_d[pg * G:(pg + 1) * G, :].rearrange("g j -> j g"))

            # FT[(pg,i), (t,g,j)]: per chunk, 16 row-group DMAs with large
            # (hi-lo)*G*T contiguous descriptors, spread over all queues
            ftv = ft[:].rearrange("p (t g j) -> p t g j", g=G, j=T)
            for lo, hi in _chunks(N):
                for pg in range(PG):
                    for rh in range(4):
                        r0 = pg * 32 + rh * 8
                        nc.sync.dma_start(
                            ftv[r0:r0 + 8, lo:hi, :, :],
                            ftp_d[pg, rh * 8:rh * 8 + 8, lo:hi, :, :])

            for t in range(1, N):
                cur = wpool.tile([P, G * T], F32, tag="cur")
                p_prev = (parthist[:, (t - 1) * G:t * G]
                          .unsqueeze(2).broadcast_to([P, G, T]))
                nc.vector.scalar_tensor_tensor(
                    out=cur[:].rearrange("p (g j) -> p g j", j=T),
                    in0=ftv[:, t, :, :], scalar=0.0, in1=p_prev,
                    op0=AL.bypass, op1=AL.add)
                nc.vector.tensor_reduce(
                    out=parthist[:, t * G:(t + 1) * G],
                    in_=cur[:].rearrange("p (g j) -> p g j", j=T),
                    axis=X, op=AL.max, apply_transpose=True)

            for lo, hi in _chunks(N):
                nc.sync.dma_start(out_d[:, lo * G:hi * G],
                                  parthist[:, lo * G:hi * G])

    nc.compile()
    return nc


def _pack_pieces(lengths, N):
    """Cut sequences into <=W*NCORES pieces of chain length N."""
    pieces = []
    for b in range(B):
        L, c = int(lengths[b]), 0
        while c < L:
            s0 = 0 if c == 0 else c - BURN
            own_end = min(s0 + N, L)
            pieces.append((b, s0, c, own_end))
            c = own_end
    return pieces if len(pieces) <= W * NCORES else None


def _choose_N(lengths):
    for N in range(40, S + BURN + 1):
        p = _pack_pieces(lengths, N)
        if p is not None:
            return N, p
    raise RuntimeError("packing failed")


def _run_device(featsp, trans, pieces, N, **spmd_kwargs):
    from concourse.bass_utils import run_bass_kernel_spmd

    key = ("prog", N)
    if key not in _PROGRAM_CACHE:
        _PROGRAM_CACHE.clear()
        _PROGRAM_CACHE[key] = _build_program(N)
    nc = _PROGRAM_CACHE[key]

    Sdim = featsp.shape[1]
    ftp = np.zeros((NCORES, PG, T, N, G, T), np.float32)
    p0 = np.zeros((NCORES, W, T), np.float32)
    for k, (b, s0, _, _) in enumerate(pieces):
        core, lane = k // W, k % W
        pg, g = lane // G, lane % G
        n = min(N, Sdim - s0)
        sl = featsp[b, s0:s0 + n]                      # [n, T]
        ftp[core, pg, :, :n, g, :] = trans[:, None, :] + sl[None, :, :]
        p0[core, lane] = sl[0] + trans[START]

    in_maps = [{"ftp": np.ascontiguousarray(ftp[c]),
                "part0": np.ascontiguousarray(p0[c])} for c in range(NCORES)]
    res = run_bass_kernel_spmd(nc, in_maps, list(range(NCORES)), **spmd_kwargs)
    _PROGRAM_CACHE["last_results"] = res

    # piece alpha: [piece, t, j]
    pa = np.zeros((len(pieces), N, T), np.float32)
    for c in range(NCORES):
        v = res.results[c]["parthist"].reshape(PG, 32, N, G)  # [pg, j, t, g]
        for k in range(min(W, len(pieces) - c * W)):
            pg, g = k // G, k % G
            pa[c * W + k] = v[pg, :, :, g].T
    return pa


def _exact_decode(feats, lengths, trans, bs):
    """Reference-exact decode for sequences bs (numpy fp32, same fl order)."""
    bs = np.asarray(sorted(bs))
    f = feats[bs]
    L = lengths[bs]
    nb = len(bs)
    a = np.empty((S, nb, T), np.float32)
    a[0] = f[:, 0] + trans[START][None, :]
    for t in range(1, S):
        FTt = (f[:, t, None, :] + trans[None, :, :]).astype(np.float32)
        a[t] = (FTt + a[t - 1][:, :, None]).max(axis=1)
    transT = np.ascontiguousarray(trans.T)
    ar = np.arange(nb)
    lp = a[L - 1, ar]
    ptr = (lp[:, :, None] + trans[None, :, :]).argmax(axis=1)[:, END].astype(np.int32)
    dec = np.zeros((S, nb), np.int32)
    dec[S - 1] = ptr
    p = ptr.copy()
    for k in range(S - 2, -1, -1):
        t = k + 1
        fc = f[ar, t, p]
        cc = ((fc[:, None] + transT[p]).astype(np.float32)
              + a[t - 1, ar]).astype(np.float32)
        bp = cc.argmax(axis=1).astype(np.int32)
        p = np.where(k == L - 1, ptr, np.where(k > L - 1, 0, bp)).astype(np.int32)
        dec[k] = p
    return bs, dec.T


def _host_decode(featsp, feats, lengths, trans, pieces, pa, N):
    """Assemble alpha from pieces, backtrack with near-tie flags, repair."""
    alpha = np.zeros((S, B, T), np.float32)
    flagged = set()
    nonstart = np.arange(T) != START
    for k, (b, s0, os_, oe) in enumerate(pieces):
        lo = os_ - s0
        if os_ > 0:  # seam coalescence check vs previous piece's column
            delta = pa[k, lo - 1][nonstart] - alpha[os_ - 1, b][nonstart]
            if float(delta.max() - delta.min()) > TAU_SEAM:
                flagged.add(b)
        alpha[os_:oe, b] = pa[k, lo:lo + (oe - os_)]

    bidx = np.arange(B)
    transT = np.ascontiguousarray(trans.T)
    last_part = alpha[lengths - 1, bidx]
    last_values = last_part[:, :, None] + trans[None, :, :]
    sv = np.sort(last_values[:, :, END], axis=1)
    min_gap = sv[:, -1] - sv[:, -2]
    pointer = last_values.argmax(axis=1)[:, END].astype(np.int32)
    decode = np.zeros((S, B), np.int32)
    decode[S - 1] = pointer
    ptr = pointer.copy()
    for k in range(S - 2, -1, -1):
        t = k + 1
        fcol = featsp[bidx, t, ptr]
        curcol = ((fcol[:, None] + transT[ptr]).astype(np.float32)
                  + alpha[t - 1, bidx]).astype(np.float32)
        sc = np.sort(curcol, axis=1)
        gap = sc[:, -1] - sc[:, -2]
        active = (k >= 1) & (k <= lengths - 2)
        min_gap = np.where(active, np.minimum(min_gap, gap), min_gap)
        bpcol = curcol.argmax(axis=1).astype(np.int32)
        newp = np.where(k == lengths - 1, pointer,
                        np.where(k > lengths - 1, 0, bpcol)).astype(np.int32)
        decode[k] = newp
        ptr = newp
    decode = decode.T.astype(np.int32)

    flagged |= set(np.where(min_gap < TAU_BP)[0].tolist())
    if flagged:
        bs, dec = _exact_decode(feats, lengths, trans, flagged)
        decode[bs] = dec
    return decode


def kernel(feats, mask, transitions, _spmd_kwargs=None):
    feats = np.asarray(feats, dtype=np.float32)
    mask_np = np.asarray(mask)
    trans = np.asarray(transitions, dtype=np.float32)
    lengths = mask_np.astype(np.int64).sum(axis=1)

    d = feats.max(axis=2).astype(np.float32)
    featsp = (feats - d[:, :, None]).astype(np.float32)

    N, pieces = _choose_N(lengths)
    pa = _run_device(featsp, trans, pieces, N, **(_spmd_kwargs or {}))
    return _host_decode(featsp, feats, lengths, trans, pieces, pa, N)


# revision 11
# speedup vs baseline: 1.8120x; 1.8120x over previous
"""CRF Viterbi decode on 8 Trainium2 NeuronCores.

Strategy: time-sliced data parallelism over 256 "virtual lanes".
  - The 64 sequences' forward Viterbi recurrences are cut (at runtime,
    from the mask) into <=256 contiguous time-pieces.  Each piece runs
    in one lane: 8 cores x 32 lanes/core, every lane a length-N forward
    chain (N ~ 85 instead of 511).  Pieces that start mid-sequence get a
    short speculative burn-in prefix; Viterbi argmax decisions coalesce
    within a few steps, after which the piece's partition vector equals
    the true one up to a constant + O(ulp) dust.
  - Drift-kill: the host subtracts max_j feats[b,t,j] per step from the
    features, keeping partition values O(10) instead of O(500) so fp32
    dust stays ~1e-5 and near-tie flips are essentially impossible.
  - The host pre-adds transitions: FT[i,tau,j] = fl(feat'+trans), so the
    device program is pure linear DMA + the 2-op/step DVE chain:
        cur  = fl(FT[t] + part_{t-1})           (scalar_tensor_tensor)
        part = max_i cur   (tensor_reduce, 32x32-block apply_transpose)
    with per-core layout partitions=(pg:4, i:32), free=(g:8, j:32).
  - Host reassembles alpha, checks seam coalescence, backtracks exactly
    like the reference, flags any decision whose top-2 gap is within
    dust range (tau=2e-4), and recomputes flagged sequences exactly.
    On non-degenerate inputs zero or a handful of sequences get flagged.
"""

import numpy as np

B, S, T = 64, 512, 32
NCORES = 8
P = 128
START, END = T - 2, T - 1
W = 32            # lanes (pieces) per core
G = 8             # free-dim lane groups
PG = 4            # partition lane groups (PG * G == W)
BURN = 12         # speculative burn-in steps
CH0, CH = 8, 24   # first / steady chunk of time-steps for DMA+output
TAU_BP = 2e-4     # near-tie flag threshold on backtrack decisions
TAU_SEAM = 5e-4   # seam coalescence threshold (excluding START column)

_PROGRAM_CACHE = {}


def _chunks(n, first=CH0, step=CH):
    out, lo = [], 0
    while lo < n:
        hi = min(n, lo + (first if lo == 0 else step))
        if n - hi < step // 3:  # merge tiny tail into last chunk
            hi = n
        out.append((lo, hi))
        lo = hi
    return out


def _build_program(N):
    import concourse.mybir as mybir
    from concourse import bacc, tile

    AL = mybir.AluOpType
    F32 = mybir.dt.float32
    X = mybir.AxisListType.X

    nc = bacc.Bacc("TRN2", target_bir_lowering=False, debug=False)
    fT_d = nc.dram_tensor("featsT", [PG, T, N, G], F32, kind="ExternalInput").ap()
    trans_d = nc.dram_tensor("trans", [T, T], F32, kind="ExternalInput").ap()
    p0_d = nc.dram_tensor("part0", [W, T], F32, kind="ExternalInput").ap()
    out_d = nc.dram_tensor("parthist", [P, N * G], F32, kind="ExternalOutput").ap()

    with tile.TileContext(nc) as tc:
        with (
            tc.tile_pool(name="work", bufs=2) as wpool,
            tc.tile_pool(name="const", bufs=1) as cpool,
        ):
            parthist = cpool.tile([P, N * G], F32, tag="parthist")
            transrep = cpool.tile([P, G * T], F32, tag="transrep")
            featsT = cpool.tile([P, N * G], F32, tag="featsT")

            # part0[(pg,j), g] = fl(lanefeat[pg*8+g, 0, j] + trans[START, j])
            # TRANSREP[(pg,i), (g,j)] = trans[i,j]
            # FEATST[(pg,j), (t,g)] = featsp[lane(pg,g), s0+t, j]
            for pg in range(PG):
                sl = slice(pg * 32, (pg + 1) * 32)
                nc.sync.dma_start(
                    parthist[sl, 0:G],
                    p0_d[pg * G:(pg + 1) * G, :].rearrange("g j -> j g"))
                nc.sync.dma_start(
                    transrep[sl, :].rearrange("p (g j) -> p g j", j=T),
                    trans_d.unsqueeze(1).broadcast_to([T, G, T]))
                for lo, hi in _chunks(N):
                    nc.sync.dma_start(
                        featsT[sl, lo * G:hi * G],
                        fT_d[pg, :, lo:hi, :].rearrange("j t g -> j (t g)"))

            tr_v = transrep[:].rearrange("p (g j) -> p g j", j=T)
            for t in range(1, N):
                cur = wpool.tile([P, G * T], F32, tag="cur")
                r = wpool.tile([P, G], F32, tag="r")
                p_prev = (parthist[:, (t - 1) * G:t * G]
                          .unsqueeze(2).broadcast_to([P, G, T]))
                nc.vector.scalar_tensor_tensor(
                    out=cur[:].rearrange("p (g j) -> p g j", j=T),
                    in0=tr_v, scalar=0.0, in1=p_prev,
                    op0=AL.bypass, op1=AL.add)
                nc.vector.tensor_reduce(
                    out=r[:],
                    in_=cur[:].rearrange("p (g j) -> p g j", j=T),
                    axis=X, op=AL.max, apply_transpose=True)
                nc.vector.scalar_tensor_tensor(
                    out=parthist[:, t * G:(t + 1) * G],
                    in0=r[:], scalar=0.0,
                    in1=featsT[:, t * G:(t + 1) * G],
                    op0=AL.bypass, op1=AL.add)

            for lo, hi in _chunks(N):
                nc.sync.dma_start(out_d[:, lo * G:hi * G],
                                  parthist[:, lo * G:hi * G])

    nc.compile()
    return nc


def _pack_pieces(lengths, N):
    """Cut sequences into <=W*NCORES pieces of chain length N."""
    pieces = []
    for b in range(B):
        L, c = int(lengths[b]), 0
        while c < L:
            s0 = 0 if c == 0 else c - BURN
            own_end = min(s0 + N, L)
            pieces.append((b, s0, c, own_end))
            c = own_end
    return pieces if len(pieces) <= W * NCORES else None


def _choose_N(lengths):
    for N in range(40, S + BURN + 1):
        p = _pack_pieces(lengths, N)
        if p is not None:
            return N, p
    raise RuntimeError("packing failed")


def _run_device(featsp, trans, pieces, N, **spmd_kwargs):
    from concourse.bass_utils import run_bass_kernel_spmd

    key = ("prog", N)
    if key not in _PROGRAM_CACHE:
        _PROGRAM_CACHE.clear()
        _PROGRAM_CACHE[key] = _build_program(N)
    nc = _PROGRAM_CACHE[key]

    Sdim = featsp.shape[1]
    fT = np.zeros((NCORES, PG, T, N, G), np.float32)
    p0 = np.zeros((NCORES, W, T), np.float32)
    for k, (b, s0, _, _) in enumerate(pieces):
        core, lane = k // W, k % W
        pg, g = lane // G, lane % G
        n = min(N, Sdim - s0)
        sl = featsp[b, s0:s0 + n]                      # [n, T]
        fT[core, pg, :, :n, g] = sl.T
        p0[core, lane] = sl[0] + trans[START]

    in_maps = [{"featsT": np.ascontiguousarray(fT[c]),
                "trans": np.ascontiguousarray(trans),
                "part0": np.ascontiguousarray(p0[c])} for c in range(NCORES)]
    res = run_bass_kernel_spmd(nc, in_maps, list(range(NCORES)), **spmd_kwargs)
    _PROGRAM_CACHE["last_results"] = res

    # piece alpha: [piece, t, j]
    pa = np.zeros((len(pieces), N, T), np.float32)
    for c in range(NCORES):
        v = res.results[c]["parthist"].reshape(PG, 32, N, G)  # [pg, j, t, g]
        for k in range(min(W, len(pieces) - c * W)):
            pg, g = k // G, k % G
            pa[c * W + k] = v[pg, :, :, g].T
    return pa


def _exact_decode(feats, lengths, trans, bs):
    """Reference-exact decode for sequences bs (numpy fp32, same fl order)."""
    bs = np.asarray(sorted(bs))
    f = feats[bs]
    L = lengths[bs]
    nb = len(bs)
    a = np.empty((S, nb, T), np.float32)
    a[0] = f[:, 0] + trans[START][None, :]
    for t in range(1, S):
        FTt = (f[:, t, None, :] + trans[None, :, :]).astype(np.float32)
        a[t] = (FTt + a[t - 1][:, :, None]).max(axis=1)
    transT = np.ascontiguousarray(trans.T)
    ar = np.arange(nb)
    lp = a[L - 1, ar]
    ptr = (lp[:, :, None] + trans[None, :, :]).argmax(axis=1)[:, END].astype(np.int32)
    dec = np.zeros((S, nb), np.int32)
    dec[S - 1] = ptr
    p = ptr.copy()
    for k in range(S - 2, -1, -1):
        t = k + 1
        fc = f[ar, t, p]
        cc = ((fc[:, None] + transT[p]).astype(np.float32)
              + a[t - 1, ar]).astype(np.float32)
        bp = cc.argmax(axis=1).astype(np.int32)
        p = np.where(k == L - 1, ptr, np.where(k > L - 1, 0, bp)).astype(np.int32)
        dec[k] = p
    return bs, dec.T


def _host_decode(featsp, feats, lengths, trans, pieces, pa, N):
    """Assemble alpha from pieces, backtrack with near-tie flags, repair."""
    alpha = np.zeros((S, B, T), np.float32)
    flagged = set()
    nonstart = np.arange(T) != START
    for k, (b, s0, os_, oe) in enumerate(pieces):
        lo = os_ - s0
        if os_ > 0:  # seam coalescence check vs previous piece's column
            delta = pa[k, lo - 1][nonstart] - alpha[os_ - 1, b][nonstart]
            if float(delta.max() - delta.min()) > TAU_SEAM:
                flagged.add(b)
        alpha[os_:oe, b] = pa[k, lo:lo + (oe - os_)]

    bidx = np.arange(B)
    transT = np.ascontiguousarray(trans.T)
    last_part = alpha[lengths - 1, bidx]
    last_values = last_part[:, :, None] + trans[None, :, :]
    sv = np.sort(last_values[:, :, END], axis=1)
    min_gap = sv[:, -1] - sv[:, -2]
    pointer = last_values.argmax(axis=1)[:, END].astype(np.int32)
    decode = np.zeros((S, B), np.int32)
    decode[S - 1] = pointer
    ptr = pointer.copy()
    for k in range(S - 2, -1, -1):
        t = k + 1
        fcol = featsp[bidx, t, ptr]
        curcol = ((fcol[:, None] + transT[ptr]).astype(np.float32)
                  + alpha[t - 1, bidx]).astype(np.float32)
        sc = np.sort(curcol, axis=1)
        gap = sc[:, -1] - sc[:, -2]
        active = (k >= 1) & (k <= lengths - 2)
        min_gap = np.where(active, np.minimum(min_gap, gap), min_gap)
        bpcol = curcol.argmax(axis=1).astype(np.int32)
        newp = np.where(k == lengths - 1, pointer,
                        np.where(k > lengths - 1, 0, bpcol)).astype(np.int32)
        decode[k] = newp
        ptr = newp
    decode = decode.T.astype(np.int32)

    flagged |= set(np.where(min_gap < TAU_BP)[0].tolist())
    if flagged:
        bs, dec = _exact_decode(feats, lengths, trans, flagged)
        decode[bs] = dec
    return decode


def kernel(feats, mask, transitions, _spmd_kwargs=None):
    feats = np.asarray(feats, dtype=np.float32)
    mask_np = np.asarray(mask)
    trans = np.asarray(transitions, dtype=np.float32)
    lengths = mask_np.astype(np.int64).sum(axis=1)

    d = feats.max(axis=2).astype(np.float32)
    featsp = (feats - d[:, :, None]).astype(np.float32)

    N, pieces = _choose_N(lengths)
    pa = _run_device(featsp, trans, pieces, N, **(_spmd_kwargs or {}))
    return _host_decode(featsp, feats, lengths, trans, pieces, pa, N)


# revision 14
# speedup vs baseline: 2.2861x; 1.2616x over previous
"""CRF Viterbi decode on 8 Trainium2 NeuronCores.

Strategy: time-sliced data parallelism over 256 "virtual lanes".
  - The 64 sequences' forward Viterbi recurrences are cut (at runtime,
    from the mask) into <=256 contiguous time-pieces.  Each piece runs
    in one lane: 8 cores x 32 lanes/core, every lane a length-N forward
    chain (N ~ 85 instead of 511).  Pieces that start mid-sequence get a
    short speculative burn-in prefix; Viterbi argmax decisions coalesce
    within a few steps, after which the piece's partition vector equals
    the true one up to a constant + O(ulp) dust.
  - Drift-kill: the host subtracts max_j feats[b,t,j] per step from the
    features, keeping partition values O(10) instead of O(500) so fp32
    dust stays ~1e-5 and near-tie flips are essentially impossible.
  - The host pre-adds transitions: FT[i,tau,j] = fl(feat'+trans), so the
    device program is pure linear DMA + the 2-op/step DVE chain:
        cur  = fl(FT[t] + part_{t-1})           (scalar_tensor_tensor)
        part = max_i cur   (tensor_reduce, 32x32-block apply_transpose)
    with per-core layout partitions=(pg:4, i:32), free=(g:8, j:32).
  - Host reassembles alpha, checks seam coalescence, backtracks exactly
    like the reference, flags any decision whose top-2 gap is within
    dust range (tau=2e-4), and recomputes flagged sequences exactly.
    On non-degenerate inputs zero or a handful of sequences get flagged.
"""

import numpy as np

B, S, T = 64, 512, 32
NCORES = 8
P = 128
START, END = T - 2, T - 1
W = 64            # lanes (pieces) per core
G = 16            # free-dim lane groups
PG = 4            # partition lane groups (PG * G == W)
BURN = 8          # speculative burn-in steps
CH0, CH = 8, 16   # first / steady chunk of time-steps for DMA+output
TAU_BP = 2e-4     # near-tie flag threshold on backtrack decisions
TAU_SEAM = 5e-4   # seam coalescence threshold (excluding START column)

_PROGRAM_CACHE = {}


def _chunks(n, first=CH0, step=CH):
    out, lo = [], 0
    while lo < n:
        hi = min(n, lo + (first if lo == 0 else step))
        if n - hi < step // 3:  # merge tiny tail into last chunk
            hi = n
        out.append((lo, hi))
        lo = hi
    return out


def _build_program(N):
    import concourse.mybir as mybir
    from concourse import bacc, tile

    AL = mybir.AluOpType
    F32 = mybir.dt.float32
    X = mybir.AxisListType.X

    # packed per-row const input: [p0 (G) | transrep (G*T) | featsT (N*G)]
    CROW = G + G * T + N * G
    nc = bacc.Bacc("TRN2", target_bir_lowering=False, debug=False)
    cin_d = nc.dram_tensor("cin", [P, CROW], F32, kind="ExternalInput").ap()
    out_d = nc.dram_tensor("parthist", [P, N * G], F32, kind="ExternalOutput").ap()

    with tile.TileContext(nc) as tc:
        with (
            tc.tile_pool(name="work", bufs=2) as wpool,
            tc.tile_pool(name="const", bufs=1) as cpool,
        ):
            parthist = cpool.tile([P, N * G], F32, tag="parthist")
            cin = cpool.tile([P, CROW], F32, tag="cin")

            # one DMA per partition quadrant; all chain inputs in one shot
            for pg in range(PG):
                sl = slice(pg * 32, (pg + 1) * 32)
                nc.sync.dma_start(cin[sl, :], cin_d[sl, :])

            p0v = cin[:, 0:G]
            tr_v = cin[:, G:G + G * T].rearrange("p (g j) -> p g j", j=T)
            fT0 = G + G * T
            for t in range(1, N):
                cur = wpool.tile([P, G * T], F32, tag="cur")
                r = wpool.tile([P, G], F32, tag="r")
                prev = (p0v if t == 1
                        else parthist[:, (t - 1) * G:t * G])
                p_prev = prev.unsqueeze(2).broadcast_to([P, G, T])
                nc.vector.scalar_tensor_tensor(
                    out=cur[:].rearrange("p (g j) -> p g j", j=T),
                    in0=tr_v, scalar=0.0, in1=p_prev,
                    op0=AL.bypass, op1=AL.add)
                nc.vector.tensor_reduce(
                    out=r[:],
                    in_=cur[:].rearrange("p (g j) -> p g j", j=T),
                    axis=X, op=AL.max, apply_transpose=True)
                nc.vector.scalar_tensor_tensor(
                    out=parthist[:, t * G:(t + 1) * G],
                    in0=r[:], scalar=0.0,
                    in1=cin[:, fT0 + t * G:fT0 + (t + 1) * G],
                    op0=AL.bypass, op1=AL.add)

            for pg in range(PG):
                sl = slice(pg * 32, (pg + 1) * 32)
                nc.sync.dma_start(out_d[sl, G:], parthist[sl, G:])

    nc.compile()
    return nc


def _pack_pieces(lengths, N):
    """Cut sequences into <=W*NCORES pieces of chain length N."""
    pieces = []
    for b in range(B):
        L, c = int(lengths[b]), 0
        while c < L:
            s0 = 0 if c == 0 else c - BURN
            own_end = min(s0 + N, L)
            pieces.append((b, s0, c, own_end))
            c = own_end
    return pieces if len(pieces) <= W * NCORES else None


def _choose_N(lengths):
    for N in range(40, S + BURN + 1):
        p = _pack_pieces(lengths, N)
        if p is not None:
            return N, p
    raise RuntimeError("packing failed")


def _run_device(featsp, trans, pieces, N, **spmd_kwargs):
    from concourse.bass_utils import run_bass_kernel_spmd

    key = ("prog", N)
    if key not in _PROGRAM_CACHE:
        _PROGRAM_CACHE.clear()
        _PROGRAM_CACHE[key] = _build_program(N)
    nc = _PROGRAM_CACHE[key]

    Sdim = featsp.shape[1]
    fT = np.zeros((NCORES, PG, T, N, G), np.float32)
    p0 = np.zeros((NCORES, W, T), np.float32)
    for k, (b, s0, _, _) in enumerate(pieces):
        core, lane = k // W, k % W
        pg, g = lane // G, lane % G
        n = min(N, Sdim - s0)
        sl = featsp[b, s0:s0 + n]                      # [n, T]
        fT[core, pg, :, :n, g] = sl.T
        p0[core, lane] = sl[0] + trans[START]

    in_maps = [{"featsT": np.ascontiguousarray(fT[c]),
                "trans": np.ascontiguousarray(trans),
                "part0": np.ascontiguousarray(p0[c])} for c in range(NCORES)]
    res = run_bass_kernel_spmd(nc, in_maps, list(range(NCORES)), **spmd_kwargs)
    _PROGRAM_CACHE["last_results"] = res

    # piece alpha: [piece, t, j]
    pa = np.zeros((len(pieces), N, T), np.float32)
    for c in range(NCORES):
        v = res.results[c]["parthist"].reshape(PG, 32, N, G)  # [pg, j, t, g]
        for k in range(min(W, len(pieces) - c * W)):
            pg, g = k // G, k % G
            pa[c * W + k] = v[pg, :, :, g].T
    return pa


def _exact_decode(feats, lengths, trans, bs):
    """Reference-exact decode for sequences bs (numpy fp32, same fl order)."""
    bs = np.asarray(sorted(bs))
    f = feats[bs]
    L = lengths[bs]
    nb = len(bs)
    a = np.empty((S, nb, T), np.float32)
    a[0] = f[:, 0] + trans[START][None, :]
    for t in range(1, S):
        FTt = (f[:, t, None, :] + trans[None, :, :]).astype(np.float32)
        a[t] = (FTt + a[t - 1][:, :, None]).max(axis=1)
    transT = np.ascontiguousarray(trans.T)
    ar = np.arange(nb)
    lp = a[L - 1, ar]
    ptr = (lp[:, :, None] + trans[None, :, :]).argmax(axis=1)[:, END].astype(np.int32)
    dec = np.zeros((S, nb), np.int32)
    dec[S - 1] = ptr
    p = ptr.copy()
    for k in range(S - 2, -1, -1):
        t = k + 1
        fc = f[ar, t, p]
        cc = ((fc[:, None] + transT[p]).astype(np.float32)
              + a[t - 1, ar]).astype(np.float32)
        bp = cc.argmax(axis=1).astype(np.int32)
        p = np.where(k == L - 1, ptr, np.where(k > L - 1, 0, bp)).astype(np.int32)
        dec[k] = p
    return bs, dec.T


def _host_decode(featsp, feats, lengths, trans, pieces, pa, N):
    """Assemble alpha from pieces, backtrack with near-tie flags, repair."""
    alpha = np.zeros((S, B, T), np.float32)
    flagged = set()
    nonstart = np.arange(T) != START
    for k, (b, s0, os_, oe) in enumerate(pieces):
        lo = os_ - s0
        if os_ > 0:  # seam coalescence check vs previous piece's column
            delta = pa[k, lo - 1][nonstart] - alpha[os_ - 1, b][nonstart]
            if float(delta.max() - delta.min()) > TAU_SEAM:
                flagged.add(b)
        alpha[os_:oe, b] = pa[k, lo:lo + (oe - os_)]

    bidx = np.arange(B)
    transT = np.ascontiguousarray(trans.T)
    last_part = alpha[lengths - 1, bidx]
    last_values = last_part[:, :, None] + trans[None, :, :]
    sv = np.sort(last_values[:, :, END], axis=1)
    min_gap = sv[:, -1] - sv[:, -2]
    pointer = last_values.argmax(axis=1)[:, END].astype(np.int32)
    decode = np.zeros((S, B), np.int32)
    decode[S - 1] = pointer
    ptr = pointer.copy()
    for k in range(S - 2, -1, -1):
        t = k + 1
        fcol = featsp[bidx, t, ptr]
        curcol = ((fcol[:, None] + transT[ptr]).astype(np.float32)
                  + alpha[t - 1, bidx]).astype(np.float32)
        sc = np.sort(curcol, axis=1)
        gap = sc[:, -1] - sc[:, -2]
        active = (k >= 1) & (k <= lengths - 2)
        min_gap = np.where(active, np.minimum(min_gap, gap), min_gap)
        bpcol = curcol.argmax(axis=1).astype(np.int32)
        newp = np.where(k == lengths - 1, pointer,
                        np.where(k > lengths - 1, 0, bpcol)).astype(np.int32)
        decode[k] = newp
        ptr = newp
    decode = decode.T.astype(np.int32)

    flagged |= set(np.where(min_gap < TAU_BP)[0].tolist())
    if flagged:
        bs, dec = _exact_decode(feats, lengths, trans, flagged)
        decode[bs] = dec
    return decode


def kernel(feats, mask, transitions, _spmd_kwargs=None):
    feats = np.asarray(feats, dtype=np.float32)
    mask_np = np.asarray(mask)
    trans = np.asarray(transitions, dtype=np.float32)
    lengths = mask_np.astype(np.int64).sum(axis=1)

    d = feats.max(axis=2).astype(np.float32)
    featsp = (feats - d[:, :, None]).astype(np.float32)

    N, pieces = _choose_N(lengths)
    pa = _run_device(featsp, trans, pieces, N, **(_spmd_kwargs or {}))
    return _host_decode(featsp, feats, lengths, trans, pieces, pa, N)


# revision 15
# speedup vs baseline: 2.5083x; 1.0972x over previous
"""CRF Viterbi decode on 8 Trainium2 NeuronCores.

Strategy: time-sliced data parallelism over 256 "virtual lanes".
  - The 64 sequences' forward Viterbi recurrences are cut (at runtime,
    from the mask) into <=256 contiguous time-pieces.  Each piece runs
    in one lane: 8 cores x 32 lanes/core, every lane a length-N forward
    chain (N ~ 85 instead of 511).  Pieces that start mid-sequence get a
    short speculative burn-in prefix; Viterbi argmax decisions coalesce
    within a few steps, after which the piece's partition vector equals
    the true one up to a constant + O(ulp) dust.
  - Drift-kill: the host subtracts max_j feats[b,t,j] per step from the
    features, keeping partition values O(10) instead of O(500) so fp32
    dust stays ~1e-5 and near-tie flips are essentially impossible.
  - The host pre-adds transitions: FT[i,tau,j] = fl(feat'+trans), so the
    device program is pure linear DMA + the 2-op/step DVE chain:
        cur  = fl(FT[t] + part_{t-1})           (scalar_tensor_tensor)
        part = max_i cur   (tensor_reduce, 32x32-block apply_transpose)
    with per-core layout partitions=(pg:4, i:32), free=(g:8, j:32).
  - Host reassembles alpha, checks seam coalescence, backtracks exactly
    like the reference, flags any decision whose top-2 gap is within
    dust range (tau=2e-4), and recomputes flagged sequences exactly.
    On non-degenerate inputs zero or a handful of sequences get flagged.
"""

import numpy as np

B, S, T = 64, 512, 32
NCORES = 8
P = 128
START, END = T - 2, T - 1
W = 64            # lanes (pieces) per core
G = 16            # free-dim lane groups
PG = 4            # partition lane groups (PG * G == W)
BURN = 8          # speculative burn-in steps
CH0, CH = 8, 16   # first / steady chunk of time-steps for DMA+output
TAU_BP = 2e-4     # near-tie flag threshold on backtrack decisions
TAU_SEAM = 5e-4   # seam coalescence threshold (excluding START column)

_PROGRAM_CACHE = {}


def _chunks(n, first=CH0, step=CH):
    out, lo = [], 0
    while lo < n:
        hi = min(n, lo + (first if lo == 0 else step))
        if n - hi < step // 3:  # merge tiny tail into last chunk
            hi = n
        out.append((lo, hi))
        lo = hi
    return out


def _build_program(N):
    import concourse.mybir as mybir
    from concourse import bacc, tile

    AL = mybir.AluOpType
    F32 = mybir.dt.float32
    X = mybir.AxisListType.X

    # packed per-row const input: [p0 (G) | transrep (G*T) | featsT (N*G)]
    CROW = G + G * T + N * G
    nc = bacc.Bacc("TRN2", target_bir_lowering=False, debug=False)
    cin_d = nc.dram_tensor("cin", [P, CROW], F32, kind="ExternalInput").ap()
    out_d = nc.dram_tensor("parthist", [P, N * G], F32, kind="ExternalOutput").ap()

    with tile.TileContext(nc) as tc:
        with (
            tc.tile_pool(name="work", bufs=2) as wpool,
            tc.tile_pool(name="const", bufs=1) as cpool,
        ):
            parthist = cpool.tile([P, N * G], F32, tag="parthist")
            cin = cpool.tile([P, CROW], F32, tag="cin")

            # one DMA per partition quadrant; all chain inputs in one shot
            for pg in range(PG):
                sl = slice(pg * 32, (pg + 1) * 32)
                nc.sync.dma_start(cin[sl, :], cin_d[sl, :])

            p0v = cin[:, 0:G]
            tr_v = cin[:, G:G + G * T].rearrange("p (g j) -> p g j", j=T)
            fT0 = G + G * T
            for t in range(1, N):
                cur = wpool.tile([P, G * T], F32, tag="cur")
                r = wpool.tile([P, G], F32, tag="r")
                prev = (p0v if t == 1
                        else parthist[:, (t - 1) * G:t * G])
                p_prev = prev.unsqueeze(2).broadcast_to([P, G, T])
                nc.vector.scalar_tensor_tensor(
                    out=cur[:].rearrange("p (g j) -> p g j", j=T),
                    in0=tr_v, scalar=0.0, in1=p_prev,
                    op0=AL.bypass, op1=AL.add)
                nc.vector.tensor_reduce(
                    out=r[:],
                    in_=cur[:].rearrange("p (g j) -> p g j", j=T),
                    axis=X, op=AL.max, apply_transpose=True)
                nc.vector.scalar_tensor_tensor(
                    out=parthist[:, t * G:(t + 1) * G],
                    in0=r[:], scalar=0.0,
                    in1=cin[:, fT0 + t * G:fT0 + (t + 1) * G],
                    op0=AL.bypass, op1=AL.add)

            for pg in range(PG):
                sl = slice(pg * 32, (pg + 1) * 32)
                nc.sync.dma_start(out_d[sl, G:], parthist[sl, G:])

    nc.compile()
    return nc


def _pack_pieces(lengths, N):
    """Cut sequences into <=W*NCORES pieces of chain length N."""
    pieces = []
    for b in range(B):
        L, c = int(lengths[b]), 0
        while c < L:
            s0 = 0 if c == 0 else c - BURN
            own_end = min(s0 + N, L)
            pieces.append((b, s0, c, own_end))
            c = own_end
    return pieces if len(pieces) <= W * NCORES else None


def _choose_N(lengths):
    for N in range(40, S + BURN + 1):
        p = _pack_pieces(lengths, N)
        if p is not None:
            return N, p
    raise RuntimeError("packing failed")


def _run_device(featsp, trans, pieces, N, **spmd_kwargs):
    from concourse.bass_utils import run_bass_kernel_spmd

    key = ("prog", N)
    if key not in _PROGRAM_CACHE:
        _PROGRAM_CACHE.clear()
        _PROGRAM_CACHE[key] = _build_program(N)
    nc = _PROGRAM_CACHE[key]

    Sdim = featsp.shape[1]
    fT = np.zeros((NCORES, PG, T, N, G), np.float32)
    p0 = np.zeros((NCORES, PG, T, G), np.float32)   # [core, pg, j, g]
    for k, (b, s0, _, _) in enumerate(pieces):
        core, lane = k // W, k % W
        pg, g = lane // G, lane % G
        n = min(N, Sdim - s0)
        sl = featsp[b, s0:s0 + n]                      # [n, T]
        fT[core, pg, :, :n, g] = sl.T
        p0[core, pg, :, g] = sl[0] + trans[START]

    # packed rows: [p0 (G) | transrep (G*T) | featsT (N*G)]
    trg = np.broadcast_to(trans[:, None, :], (T, G, T)).reshape(T, G * T)
    cin = np.empty((NCORES, P, G + G * T + N * G), np.float32)
    for c in range(NCORES):
        for pg in range(PG):
            sl = slice(pg * 32, (pg + 1) * 32)
            cin[c, sl, 0:G] = p0[c, pg]
            cin[c, sl, G:G + G * T] = trg
            cin[c, sl, G + G * T:] = fT[c, pg].reshape(T, N * G)

    in_maps = [{"cin": np.ascontiguousarray(cin[c])} for c in range(NCORES)]
    res = run_bass_kernel_spmd(nc, in_maps, list(range(NCORES)), **spmd_kwargs)
    _PROGRAM_CACHE["last_results"] = res

    # piece alpha: [piece, t, j]
    pa = np.zeros((len(pieces), N, T), np.float32)
    for c in range(NCORES):
        v = res.results[c]["parthist"].reshape(PG, 32, N, G)  # [pg, j, t, g]
        for k in range(min(W, len(pieces) - c * W)):
            pg, g = k // G, k % G
            pa[c * W + k] = v[pg, :, :, g].T
            pa[c * W + k, 0] = p0[c, pg, :, g]       # t=0 filled host-side
    return pa


def _exact_decode(feats, lengths, trans, bs):
    """Reference-exact decode for sequences bs (numpy fp32, same fl order)."""
    bs = np.asarray(sorted(bs))
    f = feats[bs]
    L = lengths[bs]
    nb = len(bs)
    a = np.empty((S, nb, T), np.float32)
    a[0] = f[:, 0] + trans[START][None, :]
    for t in range(1, S):
        FTt = (f[:, t, None, :] + trans[None, :, :]).astype(np.float32)
        a[t] = (FTt + a[t - 1][:, :, None]).max(axis=1)
    transT = np.ascontiguousarray(trans.T)
    ar = np.arange(nb)
    lp = a[L - 1, ar]
    ptr = (lp[:, :, None] + trans[None, :, :]).argmax(axis=1)[:, END].astype(np.int32)
    dec = np.zeros((S, nb), np.int32)
    dec[S - 1] = ptr
    p = ptr.copy()
    for k in range(S - 2, -1, -1):
        t = k + 1
        fc = f[ar, t, p]
        cc = ((fc[:, None] + transT[p]).astype(np.float32)
              + a[t - 1, ar]).astype(np.float32)
        bp = cc.argmax(axis=1).astype(np.int32)
        p = np.where(k == L - 1, ptr, np.where(k > L - 1, 0, bp)).astype(np.int32)
        dec[k] = p
    return bs, dec.T


def _host_decode(featsp, feats, lengths, trans, pieces, pa, N):
    """Assemble alpha from pieces, backtrack with near-tie flags, repair."""
    alpha = np.zeros((S, B, T), np.float32)
    flagged = set()
    nonstart = np.arange(T) != START
    for k, (b, s0, os_, oe) in enumerate(pieces):
        lo = os_ - s0
        if os_ > 0:  # seam coalescence check vs previous piece's column
            delta = pa[k, lo - 1][nonstart] - alpha[os_ - 1, b][nonstart]
            if float(delta.max() - delta.min()) > TAU_SEAM:
                flagged.add(b)
        alpha[os_:oe, b] = pa[k, lo:lo + (oe - os_)]

    bidx = np.arange(B)
    transT = np.ascontiguousarray(trans.T)
    last_part = alpha[lengths - 1, bidx]
    last_values = last_part[:, :, None] + trans[None, :, :]
    sv = np.sort(last_values[:, :, END], axis=1)
    min_gap = sv[:, -1] - sv[:, -2]
    pointer = last_values.argmax(axis=1)[:, END].astype(np.int32)
    decode = np.zeros((S, B), np.int32)
    decode[S - 1] = pointer
    ptr = pointer.copy()
    for k in range(S - 2, -1, -1):
        t = k + 1
        fcol = featsp[bidx, t, ptr]
        curcol = ((fcol[:, None] + transT[ptr]).astype(np.float32)
                  + alpha[t - 1, bidx]).astype(np.float32)
        sc = np.sort(curcol, axis=1)
        gap = sc[:, -1] - sc[:, -2]
        active = (k >= 1) & (k <= lengths - 2)
        min_gap = np.where(active, np.minimum(min_gap, gap), min_gap)
        bpcol = curcol.argmax(axis=1).astype(np.int32)
        newp = np.where(k == lengths - 1, pointer,
                        np.where(k > lengths - 1, 0, bpcol)).astype(np.int32)
        decode[k] = newp
        ptr = newp
    decode = decode.T.astype(np.int32)

    flagged |= set(np.where(min_gap < TAU_BP)[0].tolist())
    if flagged:
        bs, dec = _exact_decode(feats, lengths, trans, flagged)
        decode[bs] = dec
    return decode


def kernel(feats, mask, transitions, _spmd_kwargs=None):
    feats = np.asarray(feats, dtype=np.float32)
    mask_np = np.asarray(mask)
    trans = np.asarray(transitions, dtype=np.float32)
    lengths = mask_np.astype(np.int64).sum(axis=1)

    d = feats.max(axis=2).astype(np.float32)
    featsp = (feats - d[:, :, None]).astype(np.float32)

    N, pieces = _choose_N(lengths)
    pa = _run_device(featsp, trans, pieces, N, **(_spmd_kwargs or {}))
    return _host_decode(featsp, feats, lengths, trans, pieces, pa, N)
